# revision 1
# baseline (speedup 1.0000x reference)
"""GRU-cell-variant kernel for Trainium2, data-parallel over batch on 8 cores.

Reference (per batch row b, hidden size H=1024):
    gates = sigmoid(x @ W_ih + b_ih + h @ W_hh + b_hh)   # [B, 2H]
    z, r  = gates[:, :H], gates[:, H:]
    cand  = tanh(x @ W_c + b_c + r * (h @ W_hc + b_hc))
    out   = (1 - z) * h + z * cand

Design:
  - 8-way batch shard (1024 rows/core), weights replicated. No collectives.
  - Everything on-chip is computed TRANSPOSED: out.T[o, b]. That way weight
    tiles [k, o] load naturally as the stationary operand, host-pre-transposed
    x.T / h.T serve as the moving operand, and all biases are per-partition
    (free bias-add on the ACT engine).
  - Matmuls in fp16 (1 cycle/row on the PE) with fp32 PSUM accumulation;
    elementwise math and h-residual in fp32.
  - Host packs weights/activations into the exact SBUF layouts so every DMA
    is a dense 2D copy with >=2KB per-partition lines.
"""

import numpy as np

import concourse.bass as bass
import concourse.mybir as mybir
import concourse.tile as tile
from concourse import bacc
from concourse.bass_utils import run_bass_kernel_spmd

N_CORES = 8
B = 8192
H = 1024
BL = B // N_CORES  # batch rows per core
P = 128
KC = H // P  # 8 contraction chunks of 128 per 1024-wide operand
NJ = H // P  # 8 hidden-dim tiles
NB = BL // 512  # 2 moving halves of 512 batch columns

F16 = mybir.dt.float16
F32 = mybir.dt.float32
AF = mybir.ActivationFunctionType
ALU = mybir.AluOpType

_CACHE = {}


def _build_program():
    nc = bacc.Bacc(
        "TRN2",
        target_bir_lowering=False,
        debug=False,
        enable_asserts=False,
        num_devices=N_CORES,
    )

    # DRAM inputs, already packed on the host into SBUF-friendly layouts.
    # xT/hT:  [p, kc*BL + b]        = x[b, kc*128 + p]           (fp16)
    # hT32:   same layout, fp32 (residual path)
    # Wg:     [p, t*2048 + kc*128 + jj] = Wg_full[kc*128+p, t*128+jj]  (fp16)
    #          t in [0,16): gate output tile; kc in [0,16): contraction over [x;h]
    # Wc/Whc: [p, j*1024 + kc*128 + jj] = W[kc*128+p, j*128+jj]  (fp16)
    # bg:     [p, t] = (b_ih+b_hh)[t*128+p]; bc/bhc analogous.
    xT = nc.dram_tensor("xT", [P, KC * BL], F16, kind="ExternalInput").ap()
    hT = nc.dram_tensor("hT", [P, KC * BL], F16, kind="ExternalInput").ap()
    hT32 = nc.dram_tensor("hT32", [P, NJ * BL], F32, kind="ExternalInput").ap()
    Wg = nc.dram_tensor("Wg", [P, 16 * 2048], F16, kind="ExternalInput").ap()
    Wc = nc.dram_tensor("Wc", [P, NJ * H], F16, kind="ExternalInput").ap()
    Whc = nc.dram_tensor("Whc", [P, NJ * H], F16, kind="ExternalInput").ap()
    bg = nc.dram_tensor("bg", [P, 16], F32, kind="ExternalInput").ap()
    bc = nc.dram_tensor("bc", [P, NJ], F32, kind="ExternalInput").ap()
    bhc = nc.dram_tensor("bhc", [P, NJ], F32, kind="ExternalInput").ap()
    outT = nc.dram_tensor("outT", [P, NJ * BL], F32, kind="ExternalOutput").ap()

    with tile.TileContext(nc) as tc:
        with (
            tc.tile_pool(name="const", bufs=1) as cpool,
            tc.tile_pool(name="wg", bufs=4) as wgpool,
            tc.tile_pool(name="wsm", bufs=4) as wsmpool,
            tc.tile_pool(name="psum", bufs=8, space="PSUM") as ppool,
            tc.tile_pool(name="gates", bufs=6) as gpool,
            tc.tile_pool(name="work", bufs=10) as wpool,
        ):
            # Constants are DMA'd on the ACT ring below, interleaved with the
            # j=0 weight chunks (each DMA issue costs ~600ns of sequencer
            # time; the two HWDGE rings issue in parallel).
            bg_sb = cpool.tile([P, 16], F32, tag="bg")
            bc_sb = cpool.tile([P, NJ], F32, tag="bc")
            bhc_sb = cpool.tile([P, NJ], F32, tag="bhc")

            # Resident activations, loaded in per-kc chunks so the first
            # matmuls only wait on the first 128KB-256KB of traffic instead
            # of the full 8MB input preamble. hT32 (residual path, fp32) is
            # streamed per-j inside the loop — it isn't needed until the
            # first elementwise stage.
            xT_sb = cpool.tile([P, KC * BL], F16, tag="xT")
            hT_sb = cpool.tile([P, KC * BL], F16, tag="hT")
            hT32_sb = cpool.tile([P, NJ * BL], F32, tag="hT32")

            def gate_matmuls(psum, w_sb, b0):
                # accumulate over [x;h]: kc<8 reads xT, kc>=8 reads hT
                for kc in range(2 * KC):
                    src = xT_sb if kc < KC else hT_sb
                    off = (kc % KC) * BL + b0
                    nc.tensor.matmul(
                        psum[:],
                        lhsT=w_sb[:, kc * P : (kc + 1) * P],
                        rhs=src[:, off : off + 512],
                        start=(kc == 0),
                        stop=(kc == 2 * KC - 1),
                    )

            def cand_matmuls(psum, w_sb, src, b0):
                for kc in range(KC):
                    off = kc * BL + b0
                    nc.tensor.matmul(
                        psum[:],
                        lhsT=w_sb[:, kc * P : (kc + 1) * P],
                        rhs=src[:, off : off + 512],
                        start=(kc == 0),
                        stop=(kc == KC - 1),
                    )

            def load_wg(dst, t, chunks=1):
                cw = 2048 // chunks
                for c in range(chunks):
                    nc.sync.dma_start(
                        dst[:, c * cw : (c + 1) * cw],
                        Wg[:, t * 2048 + c * cw : t * 2048 + (c + 1) * cw],
                    )

            # 3D views for merged strided chunk loads: [p, kc, b]
            xs3 = xT_sb[:].rearrange("p (kc b) -> p kc b", kc=KC)
            xd3 = xT.rearrange("p (kc b) -> p kc b", kc=KC)
            hs3 = hT_sb[:].rearrange("p (kc b) -> p kc b", kc=KC)
            hd3 = hT.rearrange("p (kc b) -> p kc b", kc=KC)

            def load_act(dst3, src3, kc0, kc1, b0, bw, eng=None):
                (eng or nc.sync).dma_start(
                    dst3[:, kc0:kc1, b0 : b0 + bw], src3[:, kc0:kc1, b0 : b0 + bw]
                )

            for j in range(NJ):
                wz = wgpool.tile([P, 2048], F16, tag="wg")
                wr = wgpool.tile([P, 2048], F16, tag="wg")
                whc_w = wsmpool.tile([P, H], F16, tag="wsm")
                wc_w = wsmpool.tile([P, H], F16, tag="wsm")
                if j == 0:
                    # Cold-start feed across BOTH HWDGE rings so the issue
                    # streams run in parallel: activations on the sync ring,
                    # weights + constants on the ACT ring. The r-gate weights
                    # ride along early because the r matmuls reuse the same
                    # activation bytes (double PE work per DMA'd byte).
                    def wgc(dst, t, c):  # [128, 512] chunk c of gate col t
                        nc.scalar.dma_start(
                            dst[:, c * 512 : (c + 1) * 512],
                            Wg[:, t * 2048 + c * 512 : t * 2048 + (c + 1) * 512],
                        )

                    # Both rings carry need-adjacent items in parallel so
                    # neither steals HBM bandwidth from a more urgent item.
                    # sync ring: b0 activations, then b1 halves
                    load_act(xs3, xd3, 0, 4, 0, 512)   # x kc0-3 b0
                    load_act(xs3, xd3, 4, 8, 0, 512)   # x kc4-7 b0
                    load_act(hs3, hd3, 0, 4, 0, 512)   # h kc0-3 b0
                    load_act(hs3, hd3, 4, 8, 0, 512)   # h kc4-7 b0
                    load_act(xs3, xd3, 0, 4, 512, 512)  # x b1 kc0-3
                    load_act(hs3, hd3, 0, 4, 512, 512)  # h b1 kc0-3
                    # ACT ring: j0 weights + constants, then b1 second halves
                    wgc(wz, 0, 0)
                    wgc(wr, NJ, 0)
                    nc.scalar.dma_start(bg_sb[:], bg[:])
                    wgc(wz, 0, 1)
                    wgc(wr, NJ, 1)
                    nc.scalar.dma_start(bc_sb[:], bc[:])
                    nc.scalar.dma_start(bhc_sb[:], bhc[:])
                    wgc(wz, 0, 2)
                    wgc(wr, NJ, 2)
                    wgc(wz, 0, 3)
                    wgc(wr, NJ, 3)
                    nc.scalar.dma_start(whc_w[:], Whc[:, 0:H])
                    nc.scalar.dma_start(wc_w[:], Wc[:, 0:H])
                    load_act(xs3, xd3, 4, 8, 512, 512, eng=nc.scalar)  # x b1 kc4-7
                    load_act(hs3, hd3, 4, 8, 512, 512, eng=nc.scalar)  # h b1 kc4-7
                elif j == 1:
                    # split j=1 weights across the two rings
                    nc.sync.dma_start(wz[:], Wg[:, 1 * 2048 : 2 * 2048])
                    nc.scalar.dma_start(wr[:], Wg[:, (NJ + 1) * 2048 : (NJ + 2) * 2048])
                    nc.sync.dma_start(whc_w[:], Whc[:, H : 2 * H])
                    nc.scalar.dma_start(wc_w[:], Wc[:, H : 2 * H])
                else:
                    load_wg(wz, j)
                    load_wg(wr, NJ + j)
                    nc.sync.dma_start(whc_w[:], Whc[:, j * H : (j + 1) * H])
                    nc.sync.dma_start(wc_w[:], Wc[:, j * H : (j + 1) * H])
                # residual-path h (fp32) rides the second HWDGE ring (ACT)
                nc.scalar.dma_start(
                    hT32_sb[:, j * BL : (j + 1) * BL], hT32[:, j * BL : (j + 1) * BL]
                )

                for b in range(NB):
                    b0 = b * 512
                    hoff = j * BL + b0  # slice of hidden tile j in [p, j*BL+b] layout

                    pz = ppool.tile([P, 512], F32, tag="ps")
                    if j == 0 and b == 0:
                        # cold start: interleave z/r accumulation in 4-kc
                        # blocks matching the DMA bundle arrival order (PE
                        # executes its stream in order)
                        pr = ppool.tile([P, 512], F32, tag="ps")
                        for c in range(4):
                            for grp, w_sb in ((pz, wz), (pr, wr)):
                                for kc in range(4 * c, 4 * c + 4):
                                    src = xT_sb if kc < KC else hT_sb
                                    off = (kc % KC) * BL + b0
                                    nc.tensor.matmul(
                                        grp[:],
                                        lhsT=w_sb[:, kc * P : (kc + 1) * P],
                                        rhs=src[:, off : off + 512],
                                        start=(kc == 0),
                                        stop=(kc == 2 * KC - 1),
                                    )
                    else:
                        gate_matmuls(pz, wz, b0)
                        pr = None
                    z_sb = gpool.tile([P, 512], F32, tag="g")
                    nc.scalar.activation(z_sb[:], pz[:], AF.Sigmoid, bias=bg_sb[:, j : j + 1])
                    # zh = (z - 1) * h, computed off the critical path so the
                    # post-tanh chain is only mul + subtract
                    zh = wpool.tile([P, 512], F32, tag="w")
                    nc.vector.scalar_tensor_tensor(
                        zh[:], z_sb[:], 1.0, hT32_sb[:, hoff : hoff + 512],
                        ALU.subtract, ALU.mult,
                    )

                    if pr is None:
                        pr = ppool.tile([P, 512], F32, tag="ps")
                        gate_matmuls(pr, wr, b0)
                    r_sb = gpool.tile([P, 512], F32, tag="g")
                    nc.scalar.activation(
                        r_sb[:], pr[:], AF.Sigmoid, bias=bg_sb[:, NJ + j : NJ + j + 1]
                    )

                    ph = ppool.tile([P, 512], F32, tag="ps")
                    cand_matmuls(ph, whc_w, hT_sb, b0)
                    px = ppool.tile([P, 512], F32, tag="ps")
                    cand_matmuls(px, wc_w, xT_sb, b0)

                    # candidate + output blend; the very last unit is split
                    # into two 256-wide halves so the serial chain after the
                    # final matmul pipelines (shorter kernel tail)
                    def blend(lo, wd):
                        # rh = (hc + b_hc) * r   (one DVE op)
                        rh = wpool.tile([P, 512], F32, tag="w")
                        nc.vector.scalar_tensor_tensor(
                            rh[:, :wd], ph[:, lo : lo + wd], bhc_sb[:, j : j + 1],
                            r_sb[:, lo : lo + wd], ALU.add, ALU.mult,
                        )
                        s = wpool.tile([P, 512], F32, tag="w")
                        nc.vector.tensor_add(s[:, :wd], px[:, lo : lo + wd], rh[:, :wd])
                        cand = wpool.tile([P, 512], F32, tag="w")
                        nc.scalar.activation(
                            cand[:, :wd], s[:, :wd], AF.Tanh, bias=bc_sb[:, j : j + 1]
                        )
                        # out = z*cand - (z-1)*h
                        m = wpool.tile([P, 512], F32, tag="w")
                        nc.vector.tensor_mul(m[:, :wd], z_sb[:, lo : lo + wd], cand[:, :wd])
                        o_sb = wpool.tile([P, 512], F32, tag="w")
                        nc.vector.tensor_sub(o_sb[:, :wd], m[:, :wd], zh[:, lo : lo + wd])
                        nc.scalar.dma_start(
                            outT[:, hoff + lo : hoff + lo + wd], o_sb[:, :wd]
                        )

                    blend(0, 512)

    nc.compile()
    return nc


def _pack_weights(W_ih, b_ih, W_hh, b_hh, W_c, b_c, W_hc, b_hc):
    f16 = np.float16
    Wg_full = np.concatenate([W_ih, W_hh], axis=0)  # [2H, 2H] = [k, o]
    WgH = np.ascontiguousarray(
        Wg_full.reshape(16, P, 16, P).transpose(1, 2, 0, 3).reshape(P, 16 * 2048)
    ).astype(f16)
    WcH = np.ascontiguousarray(
        W_c.reshape(KC, P, NJ, P).transpose(1, 2, 0, 3).reshape(P, NJ * H)
    ).astype(f16)
    WhcH = np.ascontiguousarray(
        W_hc.reshape(KC, P, NJ, P).transpose(1, 2, 0, 3).reshape(P, NJ * H)
    ).astype(f16)
    bgH = np.ascontiguousarray((b_ih + b_hh).reshape(16, P).T).astype(np.float32)
    bcH = np.ascontiguousarray(b_c.reshape(NJ, P).T).astype(np.float32)
    bhcH = np.ascontiguousarray(b_hc.reshape(NJ, P).T).astype(np.float32)
    return WgH, WcH, WhcH, bgH, bcH, bhcH


def _pack_acts(a, dtype):
    # [BL, H] -> [p, kc*BL + b] with a[b, kc*128+p]
    return np.ascontiguousarray(
        a.T.reshape(KC, P, BL).transpose(1, 0, 2).reshape(P, KC * BL)
    ).astype(dtype)


def kernel(input, hx, W_ih, b_ih, W_hh, b_hh, W_c, b_c, W_hc, b_hc):
    input = np.asarray(input, np.float32)
    hx = np.asarray(hx, np.float32)
    if "nc" not in _CACHE:
        _CACHE["nc"] = _build_program()
    nc = _CACHE["nc"]

    WgH, WcH, WhcH, bgH, bcH, bhcH = _pack_weights(
        np.asarray(W_ih, np.float32), np.asarray(b_ih, np.float32),
        np.asarray(W_hh, np.float32), np.asarray(b_hh, np.float32),
        np.asarray(W_c, np.float32), np.asarray(b_c, np.float32),
        np.asarray(W_hc, np.float32), np.asarray(b_hc, np.float32),
    )

    in_maps = []
    for i in range(N_CORES):
        xs = input[i * BL : (i + 1) * BL]
        hs = hx[i * BL : (i + 1) * BL]
        in_maps.append(
            {
                "xT": _pack_acts(xs, np.float16),
                "hT": _pack_acts(hs, np.float16),
                "hT32": _pack_acts(hs, np.float32),
                "Wg": WgH,
                "Wc": WcH,
                "Whc": WhcH,
                "bg": bgH,
                "bc": bcH,
                "bhc": bhcH,
            }
        )

    res = run_bass_kernel_spmd(nc, in_maps, core_ids=list(range(N_CORES)))
    out = np.empty((B, H), np.float32)
    for i, r in enumerate(res.results):
        o = r["outT"].reshape(P, NJ, BL).transpose(2, 1, 0).reshape(BL, H)
        out[i * BL : (i + 1) * BL] = o
    return out



# revision 2
# speedup vs baseline: 1.3420x; 1.3420x over previous
"""GRU-cell-variant kernel for Trainium2, data-parallel over batch on 8 cores.

Reference (per batch row b, hidden size H=1024):
    gates = sigmoid(x @ W_ih + b_ih + h @ W_hh + b_hh)   # [B, 2H]
    z, r  = gates[:, :H], gates[:, H:]
    cand  = tanh(x @ W_c + b_c + r * (h @ W_hc + b_hc))
    out   = (1 - z) * h + z * cand

Design:
  - 8-way batch shard (1024 rows/core), weights replicated. No collectives.
  - Everything on-chip is computed TRANSPOSED: out.T[o, b]. Weight tiles
    [k, o] load as the stationary operand, host-pre-transposed x.T / h.T
    are the moving operand, and biases are per-partition (free on ACT).
  - Gate matmuls (2/3 of the FLOPs) run in fp8 e4m3 with DoubleRow perf
    mode (K=256 per instruction, 2 MACs/cell/cycle). Gate weights are
    pre-scaled by 64 on the host so the whole weight distribution sits in
    e4m3's normal range (min normal 2^-6, |w|<=2^-5); the sigmoid's input
    scale of 1/64 undoes it for free. Sigmoid's flat tails absorb the fp8
    quantization noise; measured end-to-end rel err ~1.2e-2 (budget 2e-2).
  - Candidate matmuls stay fp16 (tanh passes error straight through, so
    fp8 there would roughly double the error for only ~13% more speed).
  - Elementwise math in fp16 (2x DVE rate), final out stored fp16.
  - Host packs weights/activations into exact SBUF layouts so every DMA
    is a dense 2D/3D copy with large per-partition lines.
"""

import numpy as np
import ml_dtypes

import concourse.bass as bass
import concourse.mybir as mybir
import concourse.tile as tile
from concourse import bacc
from concourse.bass_utils import run_bass_kernel_spmd

N_CORES = 8
B = 8192
H = 1024
BL = B // N_CORES  # batch rows per core
P = 128
KC = H // P  # 8 contraction chunks of 128 per 1024-wide operand
NJ = H // P  # 8 hidden-dim tiles
WSCALE = 64.0  # gate-weight pre-scale (undone by sigmoid input scale)

E4 = ml_dtypes.float8_e4m3
F8 = mybir.dt.float8e4
F16 = mybir.dt.float16
F32 = mybir.dt.float32
AF = mybir.ActivationFunctionType
ALU = mybir.AluOpType
DR = mybir.MatmulPerfMode.DoubleRow

_CACHE = {}


def _build_program():
    nc = bacc.Bacc(
        "TRN2",
        target_bir_lowering=False,
        debug=False,
        enable_asserts=False,
        num_devices=N_CORES,
    )

    # DRAM inputs, packed on the host into SBUF-friendly layouts.
    # x8/h8:   [p, kc*BL + b] = x[b, kc*128 + p]                 (e4m3)
    # x16/h16: same layout                                        (fp16)
    # Wzr: [p, j*4096 + g*2048 + kc*128 + jj]
    #        = 64 * Wg_full[kc*128 + p, (8*g + j)*128 + jj]       (e4m3)
    #      g=0: z-gate tile j, g=1: r-gate tile j; kc in [0,16) over [x;h]
    # Wcc: [p, j*2048 + m*1024 + kc*128 + jj] = W[kc*128+p, j*128+jj]
    #      m=0: W_c, m=1: W_hc                                    (fp16)
    # bias: [p, 0:16] = (b_ih+b_hh)[t*128+p]; [p,16:24] = b_c; [p,24:32] = b_hc
    x8 = nc.dram_tensor("x8", [P, KC * BL], F8, kind="ExternalInput").ap()
    h8 = nc.dram_tensor("h8", [P, KC * BL], F8, kind="ExternalInput").ap()
    x16 = nc.dram_tensor("x16", [P, KC * BL], F16, kind="ExternalInput").ap()
    h16 = nc.dram_tensor("h16", [P, KC * BL], F16, kind="ExternalInput").ap()
    Wzr = nc.dram_tensor("Wzr", [P, NJ * 4096], F8, kind="ExternalInput").ap()
    Wcc = nc.dram_tensor("Wcc", [P, NJ * 2048], F16, kind="ExternalInput").ap()
    bias = nc.dram_tensor("bias", [P, 32], F32, kind="ExternalInput").ap()
    outT = nc.dram_tensor("outT", [P, NJ * BL], F16, kind="ExternalOutput").ap()

    with tile.TileContext(nc) as tc:
        with (
            tc.tile_pool(name="const", bufs=1) as cpool,
            tc.tile_pool(name="wzr", bufs=2) as wzrpool,
            tc.tile_pool(name="wcc", bufs=2) as wccpool,
            tc.tile_pool(name="psum", bufs=8, space="PSUM") as ppool,
            tc.tile_pool(name="gates", bufs=8) as gpool,
            tc.tile_pool(name="work", bufs=12) as wpool,
        ):
            bias_sb = cpool.tile([P, 32], F32, tag="bias")
            x8_sb = cpool.tile([P, KC * BL], F8, tag="x8")
            h8_sb = cpool.tile([P, KC * BL], F8, tag="h8")
            x16_sb = cpool.tile([P, KC * BL], F16, tag="x16")
            h16_sb = cpool.tile([P, KC * BL], F16, tag="h16")

            # 3D views [p, kc, b] for strided chunk DMAs and DoubleRow APs
            x8s = x8_sb[:].rearrange("p (kc b) -> p kc b", kc=KC)
            h8s = h8_sb[:].rearrange("p (kc b) -> p kc b", kc=KC)
            x16s = x16_sb[:].rearrange("p (kc b) -> p kc b", kc=KC)
            h16s = h16_sb[:].rearrange("p (kc b) -> p kc b", kc=KC)
            x8d = x8.rearrange("p (kc b) -> p kc b", kc=KC)
            h8d = h8.rearrange("p (kc b) -> p kc b", kc=KC)
            x16d = x16.rearrange("p (kc b) -> p kc b", kc=KC)
            h16d = h16.rearrange("p (kc b) -> p kc b", kc=KC)

            def gate_mms(pz0, pz1, w3):
                # accumulate K=2048 over [x;h] in 8 DoubleRow chunks of 256;
                # both batch halves ride each stationary chunk so LDWEIGHTS
                # can be reused across the pair.
                for c in range(2 * KC // 2):
                    src3 = x8s if c < 4 else h8s
                    cc = 2 * c if c < 4 else 2 * (c - 4)
                    w = w3[:, 2 * c : 2 * c + 2, :]
                    for b, pz in ((0, pz0), (1, pz1)):
                        nc.tensor.matmul(
                            pz[:],
                            lhsT=w,
                            rhs=src3[:, cc : cc + 2, b * 512 : b * 512 + 512],
                            start=(c == 0),
                            stop=(c == 7),
                            perf_mode=DR,
                        )

            def cand_mms(p0, p1, w2, lo, src_sb):
                # fp16, K=1024 in 8 chunks of 128; b-inner for LDW reuse
                for kc in range(KC):
                    w = w2[:, lo + kc * P : lo + (kc + 1) * P]
                    for b, ps in ((0, p0), (1, p1)):
                        off = kc * BL + b * 512
                        nc.tensor.matmul(
                            ps[:],
                            lhsT=w,
                            rhs=src_sb[:, off : off + 512],
                            start=(kc == 0),
                            stop=(kc == KC - 1),
                        )

            for j in range(NJ):
                wzr_t = wzrpool.tile([P, 4096], F8, tag="wzr")
                wcc_t = wccpool.tile([P, 2048], F16, tag="wcc")
                if j == 0:
                    # Cold-start feed across BOTH HWDGE rings in first-use
                    # order. scalar ring: weights (+x16 later); sync ring:
                    # fp8 activations then h16 (residual + cand-h operand).
                    nc.scalar.dma_start(wzr_t[:, 0:1024], Wzr[:, 0:1024])
                    nc.sync.dma_start(x8s[:, 0:2, :], x8d[:, 0:2, :])
                    nc.sync.dma_start(x8s[:, 2:4, :], x8d[:, 2:4, :])
                    nc.scalar.dma_start(wzr_t[:, 1024:2048], Wzr[:, 1024:2048])
                    nc.sync.dma_start(x8s[:, 4:6, :], x8d[:, 4:6, :])
                    nc.sync.dma_start(x8s[:, 6:8, :], x8d[:, 6:8, :])
                    nc.scalar.dma_start(wzr_t[:, 2048:3072], Wzr[:, 2048:3072])
                    nc.scalar.dma_start(wzr_t[:, 3072:4096], Wzr[:, 3072:4096])
                    nc.sync.dma_start(h8s[:, 0:2, :], h8d[:, 0:2, :])
                    nc.sync.dma_start(h8s[:, 2:4, :], h8d[:, 2:4, :])
                    nc.scalar.dma_start(bias_sb[:], bias[:])
                    nc.sync.dma_start(h8s[:, 4:6, :], h8d[:, 4:6, :])
                    nc.sync.dma_start(h8s[:, 6:8, :], h8d[:, 6:8, :])
                    nc.scalar.dma_start(wcc_t[:], Wcc[:, 0:2048])
                    # h16: kc=j(=0) chunk first (zh residual), then the rest
                    nc.sync.dma_start(h16s[:, 0:2, :], h16d[:, 0:2, :])
                    nc.sync.dma_start(h16s[:, 2:8, :], h16d[:, 2:8, :])
                    nc.scalar.dma_start(x16s[:, 0:4, :], x16d[:, 0:4, :])
                    nc.scalar.dma_start(x16s[:, 4:8, :], x16d[:, 4:8, :])
                else:
                    nc.scalar.dma_start(wzr_t[:], Wzr[:, j * 4096 : (j + 1) * 4096])
                    nc.scalar.dma_start(wcc_t[:], Wcc[:, j * 2048 : (j + 1) * 2048])

                wz3 = wzr_t[:, 0:2048].rearrange("p (kc o) -> p kc o", kc=16)
                wr3 = wzr_t[:, 2048:4096].rearrange("p (kc o) -> p kc o", kc=16)

                # z gate (both batch halves), then residual-blend prefix
                pz0 = ppool.tile([P, 512], F32, tag="ps")
                pz1 = ppool.tile([P, 512], F32, tag="ps")
                gate_mms(pz0, pz1, wz3)
                z = []
                zh = []
                for b, pz in ((0, pz0), (1, pz1)):
                    z_sb = gpool.tile([P, 512], F16, tag="g")
                    nc.scalar.activation(
                        z_sb[:], pz[:], AF.Sigmoid,
                        bias=bias_sb[:, j : j + 1], scale=1.0 / WSCALE,
                    )
                    hslice = h16_sb[:, j * BL + b * 512 : j * BL + b * 512 + 512]
                    zh_sb = wpool.tile([P, 512], F16, tag="w")
                    nc.vector.scalar_tensor_tensor(
                        zh_sb[:], z_sb[:], 1.0, hslice, ALU.subtract, ALU.mult
                    )
                    z.append(z_sb)
                    zh.append(zh_sb)

                # r gate
                pr0 = ppool.tile([P, 512], F32, tag="ps")
                pr1 = ppool.tile([P, 512], F32, tag="ps")
                gate_mms(pr0, pr1, wr3)
                r = []
                for b, pr in ((0, pr0), (1, pr1)):
                    r_sb = gpool.tile([P, 512], F16, tag="g")
                    nc.scalar.activation(
                        r_sb[:], pr[:], AF.Sigmoid,
                        bias=bias_sb[:, 8 + j : 9 + j], scale=1.0 / WSCALE,
                    )
                    r.append(r_sb)

                # candidate path: h@W_hc then x@W_c (fp16)
                ph0 = ppool.tile([P, 512], F32, tag="ps")
                ph1 = ppool.tile([P, 512], F32, tag="ps")
                cand_mms(ph0, ph1, wcc_t, 1024, h16_sb)
                px0 = ppool.tile([P, 512], F32, tag="ps")
                px1 = ppool.tile([P, 512], F32, tag="ps")
                cand_mms(px0, px1, wcc_t, 0, x16_sb)

                for b, ph, px in ((0, ph0, px0), (1, ph1, px1)):
                    # rh = (hc + b_hc) * r   (one DVE op)
                    rh = wpool.tile([P, 512], F16, tag="w")
                    nc.vector.scalar_tensor_tensor(
                        rh[:], ph[:], bias_sb[:, 24 + j : 25 + j], r[b][:],
                        ALU.add, ALU.mult,
                    )
                    s = wpool.tile([P, 512], F16, tag="w")
                    nc.vector.tensor_add(s[:], px[:], rh[:])
                    cand = wpool.tile([P, 512], F16, tag="w")
                    nc.scalar.activation(
                        cand[:], s[:], AF.Tanh, bias=bias_sb[:, 16 + j : 17 + j]
                    )
                    # out = z*cand - (z-1)*h, tail ops on the idle GpSimd
                    m = wpool.tile([P, 512], F16, tag="w")
                    nc.gpsimd.tensor_mul(m[:], z[b][:], cand[:])
                    o_sb = wpool.tile([P, 512], F16, tag="w")
                    nc.gpsimd.tensor_sub(o_sb[:], m[:], zh[b][:])
                    hoff = j * BL + b * 512
                    nc.sync.dma_start(outT[:, hoff : hoff + 512], o_sb[:])

    nc.compile()
    return nc


def _pack_weights(W_ih, b_ih, W_hh, b_hh, W_c, b_c, W_hc, b_hc):
    Wg_full = np.concatenate([W_ih, W_hh], axis=0)  # [2H, 2H] = [k, o]
    # [kc, p, t, jj] -> [p, t, kc, jj]
    wg = Wg_full.reshape(16, P, 16, P).transpose(1, 2, 0, 3)
    WzrH = np.concatenate(
        [
            np.concatenate([wg[:, j].reshape(P, 2048), wg[:, 8 + j].reshape(P, 2048)], axis=1)
            for j in range(NJ)
        ],
        axis=1,
    )
    WzrH = np.ascontiguousarray(WzrH * WSCALE).astype(E4)
    wc = W_c.reshape(KC, P, NJ, P).transpose(1, 2, 0, 3)
    whc = W_hc.reshape(KC, P, NJ, P).transpose(1, 2, 0, 3)
    WccH = np.concatenate(
        [
            np.concatenate([wc[:, j].reshape(P, 1024), whc[:, j].reshape(P, 1024)], axis=1)
            for j in range(NJ)
        ],
        axis=1,
    )
    WccH = np.ascontiguousarray(WccH).astype(np.float16)
    biasH = np.empty((P, 32), np.float32)
    biasH[:, 0:16] = (b_ih + b_hh).reshape(16, P).T
    biasH[:, 16:24] = b_c.reshape(NJ, P).T
    biasH[:, 24:32] = b_hc.reshape(NJ, P).T
    return WzrH, WccH, np.ascontiguousarray(biasH)


def _pack_acts(a, dtype):
    # [BL, H] -> [p, kc*BL + b] with a[b, kc*128+p]
    return np.ascontiguousarray(
        a.T.reshape(KC, P, BL).transpose(1, 0, 2).reshape(P, KC * BL)
    ).astype(dtype)


def make_in_maps(input, hx, W_ih, b_ih, W_hh, b_hh, W_c, b_c, W_hc, b_hc):
    input = np.asarray(input, np.float32)
    hx = np.asarray(hx, np.float32)
    WzrH, WccH, biasH = _pack_weights(
        np.asarray(W_ih, np.float32), np.asarray(b_ih, np.float32),
        np.asarray(W_hh, np.float32), np.asarray(b_hh, np.float32),
        np.asarray(W_c, np.float32), np.asarray(b_c, np.float32),
        np.asarray(W_hc, np.float32), np.asarray(b_hc, np.float32),
    )
    in_maps = []
    for i in range(N_CORES):
        xs = input[i * BL : (i + 1) * BL]
        hs = hx[i * BL : (i + 1) * BL]
        in_maps.append(
            {
                "x8": _pack_acts(xs, E4),
                "h8": _pack_acts(hs, E4),
                "x16": _pack_acts(xs, np.float16),
                "h16": _pack_acts(hs, np.float16),
                "Wzr": WzrH,
                "Wcc": WccH,
                "bias": biasH,
            }
        )
    return in_maps


def kernel(input, hx, W_ih, b_ih, W_hh, b_hh, W_c, b_c, W_hc, b_hc):
    if "nc" not in _CACHE:
        _CACHE["nc"] = _build_program()
    nc = _CACHE["nc"]

    in_maps = make_in_maps(
        input, hx, W_ih, b_ih, W_hh, b_hh, W_c, b_c, W_hc, b_hc
    )
    res = run_bass_kernel_spmd(nc, in_maps, core_ids=list(range(N_CORES)))
    out = np.empty((B, H), np.float32)
    for i, r in enumerate(res.results):
        o = (
            np.asarray(r["outT"])
            .astype(np.float32)
            .reshape(P, NJ, BL)
            .transpose(2, 1, 0)
            .reshape(BL, H)
        )
        out[i * BL : (i + 1) * BL] = o
    return out


# revision 3
# speedup vs baseline: 1.4222x; 1.0598x over previous
"""GRU-cell-variant kernel for Trainium2, data-parallel over batch on 8 cores.

Reference (per batch row b, hidden size H=1024):
    gates = sigmoid(x @ W_ih + b_ih + h @ W_hh + b_hh)   # [B, 2H]
    z, r  = gates[:, :H], gates[:, H:]
    cand  = tanh(x @ W_c + b_c + r * (h @ W_hc + b_hc))
    out   = (1 - z) * h + z * cand

Design:
  - 8-way batch shard (1024 rows/core), weights replicated. No collectives.
  - Everything on-chip is computed TRANSPOSED: out.T[o, b].
  - Warm PE streams one moving column per cycle regardless of dtype, so
    wall time ~ matmul count. fp8 DoubleRow packs K=256 per instruction
    (2 fp8 MACs/cell/cycle), halving instruction count for any matrix we
    can afford in e4m3: both gate matrices and W_hc (whose error is damped
    by sigmoid slope resp. r<1). W_c stays fp16 (tanh passes its error
    straight through; all-fp8 misses the 2e-2 budget). 448 matmuls total.
  - fp8 weights are pre-scaled by 64 on the host so the whole weight
    distribution sits in e4m3's normal range (min normal 2^-6, |w|<=2^-5);
    the activation input scale (sigmoid) resp. a folded 1/64 in the stt
    chain (candidate) undoes it for free. Measured rel err ~1.4e-2.
  - Software pipeline: iteration k runs gates(j=k) and candidate(j=k-1),
    so the cold start only needs the small fp8 operands + gate weights,
    and x16 (the big fp16 moving operand) has an extra 12us to arrive.
  - Three DMA channels: scalar HWDGE (weights), sync HWDGE (x8/x16/out),
    gpsimd SWDGE (h8/h16) — one HWDGE ring sustains only ~100GB/s, and
    iteration 0 needs ~2.6MB in ~8us.
  - ~44 dummy matmuls on a zeroed tile run during the DMA preamble so the
    PE's HAM clock gate is already at 2.4GHz when real data lands.
"""

import numpy as np
import ml_dtypes

import concourse.bass as bass
import concourse.mybir as mybir
import concourse.tile as tile
from concourse import bacc
from concourse.bass_utils import run_bass_kernel_spmd

N_CORES = 8
B = 8192
H = 1024
BL = B // N_CORES  # batch rows per core
P = 128
KC = H // P  # 8 contraction chunks of 128 per 1024-wide operand
NJ = H // P  # 8 hidden-dim tiles
WSCALE = 64.0  # fp8 weight pre-scale (undone downstream)

E4 = ml_dtypes.float8_e4m3
F8 = mybir.dt.float8e4
F16 = mybir.dt.float16
F32 = mybir.dt.float32
AF = mybir.ActivationFunctionType
ALU = mybir.AluOpType
DR = mybir.MatmulPerfMode.DoubleRow

_CACHE = {}


def _build_program():
    nc = bacc.Bacc(
        "TRN2",
        target_bir_lowering=False,
        debug=False,
        enable_asserts=False,
        num_devices=N_CORES,
    )

    # DRAM inputs, packed on the host into SBUF-friendly layouts.
    # x8/h8:   [p, kc*BL + b] = x[b, kc*128 + p]                 (e4m3)
    # x16/h16: same layout                                        (fp16)
    # Wq8: per j block of 5120 cols: [wz (2048) | wr (2048) | whc (1024)]
    #      wz/wr: [p, kc*128 + jj] = 64*Wg_full[kc*128+p, (g*8+j)*128+jj]
    #      whc:   [p, kc*128 + jj] = 64*W_hc[kc*128+p, j*128+jj]  (e4m3)
    # Wc16: [p, j*1024 + kc*128 + jj] = W_c[kc*128+p, j*128+jj]   (fp16)
    # bias: [p, 0:16] = (b_ih+b_hh)[t*128+p]; [p,16:24] = b_c;
    #       [p,24:32] = 64*b_hc
    x8 = nc.dram_tensor("x8", [P, KC * BL], F8, kind="ExternalInput").ap()
    h8 = nc.dram_tensor("h8", [P, KC * BL], F8, kind="ExternalInput").ap()
    x16 = nc.dram_tensor("x16", [P, KC * BL], F16, kind="ExternalInput").ap()
    h16 = nc.dram_tensor("h16", [P, KC * BL], F16, kind="ExternalInput").ap()
    Wq8 = nc.dram_tensor("Wq8", [P, NJ * 5120], F8, kind="ExternalInput").ap()
    Wc16 = nc.dram_tensor("Wc16", [P, NJ * 1024], F16, kind="ExternalInput").ap()
    bias = nc.dram_tensor("bias", [P, 32], F32, kind="ExternalInput").ap()
    outT = nc.dram_tensor("outT", [P, NJ * BL], F16, kind="ExternalOutput").ap()

    with tile.TileContext(nc) as tc:
        with (
            tc.tile_pool(name="const", bufs=1) as cpool,
            tc.tile_pool(name="wq", bufs=3) as wqpool,
            tc.tile_pool(name="wc", bufs=2) as wcpool,
            tc.tile_pool(name="psum", bufs=8, space="PSUM") as ppool,
            tc.tile_pool(name="gates", bufs=14) as gpool,
            tc.tile_pool(name="work", bufs=14) as wpool,
        ):
            bias_sb = cpool.tile([P, 32], F32, tag="bias")
            x8_sb = cpool.tile([P, KC * BL], F8, tag="x8")
            h8_sb = cpool.tile([P, KC * BL], F8, tag="h8")
            x16_sb = cpool.tile([P, KC * BL], F16, tag="x16")
            h16_sb = cpool.tile([P, KC * BL], F16, tag="h16")
            warm = cpool.tile([P, 640], F16, tag="warm")

            # 3D views [p, kc, b] for strided chunk DMAs and DoubleRow APs
            x8s = x8_sb[:].rearrange("p (kc b) -> p kc b", kc=KC)
            h8s = h8_sb[:].rearrange("p (kc b) -> p kc b", kc=KC)
            x16s = x16_sb[:].rearrange("p (kc b) -> p kc b", kc=KC)
            h16s = h16_sb[:].rearrange("p (kc b) -> p kc b", kc=KC)
            x8d = x8.rearrange("p (kc b) -> p kc b", kc=KC)
            h8d = h8.rearrange("p (kc b) -> p kc b", kc=KC)
            x16d = x16.rearrange("p (kc b) -> p kc b", kc=KC)
            h16d = h16.rearrange("p (kc b) -> p kc b", kc=KC)

            # PE warm-up: dummy matmuls on a zeroed tile keep the HAM
            # activity monitor busy through the DMA preamble so real
            # matmuls start at 2.4GHz instead of 1.2GHz.
            nc.vector.memset(warm[:], 0.0)
            pw = ppool.tile([P, 512], F32, tag="ps")
            for _ in range(44):
                nc.tensor.matmul(
                    pw[:], lhsT=warm[:, 0:128], rhs=warm[:, 128:640],
                    start=True, stop=True,
                )

            def gate_mms(pz0, pz1, w3):
                # K=2048 over [x;h] in 8 DoubleRow chunks of 256; both
                # batch halves ride each stationary chunk.
                for c in range(8):
                    src3 = x8s if c < 4 else h8s
                    cc = 2 * c if c < 4 else 2 * (c - 4)
                    w = w3[:, 2 * c : 2 * c + 2, :]
                    for b, pz in ((0, pz0), (1, pz1)):
                        nc.tensor.matmul(
                            pz[:],
                            lhsT=w,
                            rhs=src3[:, cc : cc + 2, b * 512 : b * 512 + 512],
                            start=(c == 0),
                            stop=(c == 7),
                            perf_mode=DR,
                        )

            def ch_mms(ph, w3, bsl):
                # h @ W_hc for one batch half: K=1024 in 4 DoubleRow chunks
                for c in range(4):
                    nc.tensor.matmul(
                        ph[:],
                        lhsT=w3[:, 2 * c : 2 * c + 2, :],
                        rhs=h8s[:, 2 * c : 2 * c + 2, bsl],
                        start=(c == 0),
                        stop=(c == 3),
                        perf_mode=DR,
                    )

            def cx_mms(px, wc_t, jc, bsl0):
                # x @ W_c for one batch half: fp16, K=1024 in 8 chunks
                for kc in range(KC):
                    nc.tensor.matmul(
                        px[:],
                        lhsT=wc_t[:, kc * P : (kc + 1) * P],
                        rhs=x16_sb[:, kc * BL + bsl0 : kc * BL + bsl0 + 512],
                        start=(kc == 0),
                        stop=(kc == KC - 1),
                    )

            wq_tiles = {}
            wc_tiles = {}
            zs, rs, zhs = {}, {}, {}

            def eltwise(jc, b, ph, px, lo, wd):
                # candidate + blend for a [lo:lo+wd) slice of batch half b
                r_sb, z_sb, zh_sb = rs[jc][b], zs[jc][b], zhs[jc][b]
                rh = wpool.tile([P, 512], F16, tag="w")
                nc.vector.scalar_tensor_tensor(
                    rh[:, :wd], ph[:, lo : lo + wd],
                    bias_sb[:, 24 + jc : 25 + jc], r_sb[:, lo : lo + wd],
                    ALU.add, ALU.mult,
                )
                s = wpool.tile([P, 512], F16, tag="w")
                nc.vector.scalar_tensor_tensor(
                    s[:, :wd], rh[:, :wd], 1.0 / WSCALE, px[:, lo : lo + wd],
                    ALU.mult, ALU.add,
                )
                cand = wpool.tile([P, 512], F16, tag="w")
                nc.scalar.activation(
                    cand[:, :wd], s[:, :wd], AF.Tanh,
                    bias=bias_sb[:, 16 + jc : 17 + jc],
                )
                m = wpool.tile([P, 512], F16, tag="w")
                nc.vector.tensor_mul(
                    m[:, :wd], z_sb[:, lo : lo + wd], cand[:, :wd]
                )
                o_sb = wpool.tile([P, 512], F16, tag="w")
                nc.vector.tensor_sub(o_sb[:, :wd], m[:, :wd], zh_sb[:, lo : lo + wd])
                hoff = jc * BL + b * 512 + lo
                nc.sync.dma_start(outT[:, hoff : hoff + wd], o_sb[:, :wd])

            for k in range(NJ + 1):
                # ---- gates for j = k (+ weight/operand streaming) ----
                if k < NJ:
                    j = k
                    wq_t = wqpool.tile([P, 5120], F8, tag="wq")
                    wq_tiles[j] = wq_t
                    if k == 0:
                        # scalar ring: gate weights in chunks, then constants
                        nc.scalar.dma_start(wq_t[:, 0:1024], Wq8[:, 0:1024])
                        nc.scalar.dma_start(wq_t[:, 1024:2048], Wq8[:, 1024:2048])
                        nc.scalar.dma_start(wq_t[:, 2048:3072], Wq8[:, 2048:3072])
                        nc.scalar.dma_start(wq_t[:, 3072:4096], Wq8[:, 3072:4096])
                        nc.scalar.dma_start(wq_t[:, 4096:5120], Wq8[:, 4096:5120])
                        nc.scalar.dma_start(bias_sb[:], bias[:])
                        # sync ring: x8 chunks, then x16 batch-half-major
                        nc.sync.dma_start(x8s[:, 0:2, :], x8d[:, 0:2, :])
                        nc.sync.dma_start(x8s[:, 2:4, :], x8d[:, 2:4, :])
                        nc.sync.dma_start(x8s[:, 4:6, :], x8d[:, 4:6, :])
                        nc.sync.dma_start(x8s[:, 6:8, :], x8d[:, 6:8, :])
                        nc.sync.dma_start(x16s[:, :, 0:512], x16d[:, :, 0:512])
                        nc.sync.dma_start(x16s[:, :, 512:1024], x16d[:, :, 512:1024])
                        # gpsimd SWDGE ring: h8 chunks, then h16 per-j stream
                        nc.gpsimd.dma_start(h8s[:, 0:2, :], h8d[:, 0:2, :])
                        nc.gpsimd.dma_start(h8s[:, 2:4, :], h8d[:, 2:4, :])
                        nc.gpsimd.dma_start(h8s[:, 4:6, :], h8d[:, 4:6, :])
                        nc.gpsimd.dma_start(h8s[:, 6:8, :], h8d[:, 6:8, :])
                        nc.gpsimd.dma_start(h16s[:, 0, :], h16d[:, 0, :])
                    else:
                        nc.scalar.dma_start(
                            wq_t[:], Wq8[:, j * 5120 : (j + 1) * 5120]
                        )
                        nc.gpsimd.dma_start(h16s[:, j, :], h16d[:, j, :])
                    # W_c tile for THIS j rides ahead; used next iteration
                    wc_t = wcpool.tile([P, 1024], F16, tag="wc")
                    wc_tiles[j] = wc_t
                    nc.scalar.dma_start(wc_t[:], Wc16[:, j * 1024 : (j + 1) * 1024])

                    wz3 = wq_t[:, 0:2048].rearrange("p (kc o) -> p kc o", kc=16)
                    wr3 = wq_t[:, 2048:4096].rearrange("p (kc o) -> p kc o", kc=16)

                    pz0 = ppool.tile([P, 512], F32, tag="ps")
                    pz1 = ppool.tile([P, 512], F32, tag="ps")
                    gate_mms(pz0, pz1, wz3)
                    zs[j], zhs[j] = [], []
                    for b, pz in ((0, pz0), (1, pz1)):
                        z_sb = gpool.tile([P, 512], F16, tag="g")
                        nc.scalar.activation(
                            z_sb[:], pz[:], AF.Sigmoid,
                            bias=bias_sb[:, j : j + 1], scale=1.0 / WSCALE,
                        )
                        hsl = h16_sb[:, j * BL + b * 512 : j * BL + b * 512 + 512]
                        zh_sb = gpool.tile([P, 512], F16, tag="g")
                        nc.vector.scalar_tensor_tensor(
                            zh_sb[:], z_sb[:], 1.0, hsl, ALU.subtract, ALU.mult
                        )
                        zs[j].append(z_sb)
                        zhs[j].append(zh_sb)

                    pr0 = ppool.tile([P, 512], F32, tag="ps")
                    pr1 = ppool.tile([P, 512], F32, tag="ps")
                    gate_mms(pr0, pr1, wr3)
                    rs[j] = []
                    for b, pr in ((0, pr0), (1, pr1)):
                        r_sb = gpool.tile([P, 512], F16, tag="g")
                        nc.scalar.activation(
                            r_sb[:], pr[:], AF.Sigmoid,
                            bias=bias_sb[:, 8 + j : 9 + j], scale=1.0 / WSCALE,
                        )
                        rs[j].append(r_sb)

                # ---- candidate + blend for jc = k-1 ----
                if k >= 1:
                    jc = k - 1
                    whc3 = wq_tiles[jc][:, 4096:5120].rearrange(
                        "p (kc o) -> p kc o", kc=KC
                    )
                    wc_t = wc_tiles[jc]
                    if k < NJ:
                        # interleave both halves (shared stationary chunks)
                        ph0 = ppool.tile([P, 512], F32, tag="ps")
                        ph1 = ppool.tile([P, 512], F32, tag="ps")
                        for c in range(4):
                            w = whc3[:, 2 * c : 2 * c + 2, :]
                            for b, ph in ((0, ph0), (1, ph1)):
                                nc.tensor.matmul(
                                    ph[:], lhsT=w,
                                    rhs=h8s[:, 2 * c : 2 * c + 2,
                                            b * 512 : b * 512 + 512],
                                    start=(c == 0), stop=(c == 3),
                                    perf_mode=DR,
                                )
                        px0 = ppool.tile([P, 512], F32, tag="ps")
                        px1 = ppool.tile([P, 512], F32, tag="ps")
                        for kc in range(KC):
                            w = wc_t[:, kc * P : (kc + 1) * P]
                            for b, px in ((0, px0), (1, px1)):
                                nc.tensor.matmul(
                                    px[:], lhsT=w,
                                    rhs=x16_sb[:, kc * BL + b * 512 :
                                               kc * BL + b * 512 + 512],
                                    start=(kc == 0), stop=(kc == KC - 1),
                                )
                        eltwise(jc, 0, ph0, px0, 0, 512)
                        eltwise(jc, 1, ph1, px1, 0, 512)
                    else:
                        # final iteration: batch-half-serial so b0's chain
                        # hides under b1's matmuls; 256-wide chunks at the
                        # very end to pipeline DVE/ACT and the out DMA.
                        for b in range(2):
                            ph = ppool.tile([P, 512], F32, tag="ps")
                            ch_mms(ph, whc3, slice(b * 512, b * 512 + 512))
                            px = ppool.tile([P, 512], F32, tag="ps")
                            cx_mms(px, wc_t, jc, b * 512)
                            eltwise(jc, b, ph, px, 0, 256)
                            eltwise(jc, b, ph, px, 256, 256)

    nc.compile()
    return nc


def _pack_weights(W_ih, b_ih, W_hh, b_hh, W_c, b_c, W_hc, b_hc):
    Wg_full = np.concatenate([W_ih, W_hh], axis=0)  # [2H, 2H] = [k, o]
    # [kc, p, t, jj] -> [p, t, kc, jj]
    wg = Wg_full.reshape(16, P, 16, P).transpose(1, 2, 0, 3)
    whc = W_hc.reshape(KC, P, NJ, P).transpose(1, 2, 0, 3)
    Wq8H = np.concatenate(
        [
            np.concatenate(
                [
                    wg[:, j].reshape(P, 2048),
                    wg[:, 8 + j].reshape(P, 2048),
                    whc[:, j].reshape(P, 1024),
                ],
                axis=1,
            )
            for j in range(NJ)
        ],
        axis=1,
    )
    Wq8H = np.ascontiguousarray(Wq8H * WSCALE).astype(E4)
    wc = W_c.reshape(KC, P, NJ, P).transpose(1, 2, 0, 3)
    Wc16H = np.ascontiguousarray(
        np.concatenate([wc[:, j].reshape(P, 1024) for j in range(NJ)], axis=1)
    ).astype(np.float16)
    biasH = np.empty((P, 32), np.float32)
    biasH[:, 0:16] = (b_ih + b_hh).reshape(16, P).T
    biasH[:, 16:24] = b_c.reshape(NJ, P).T
    biasH[:, 24:32] = WSCALE * b_hc.reshape(NJ, P).T
    return Wq8H, Wc16H, np.ascontiguousarray(biasH)


def _pack_acts(a, dtype):
    # [BL, H] -> [p, kc*BL + b] with a[b, kc*128+p]
    return np.ascontiguousarray(
        a.T.reshape(KC, P, BL).transpose(1, 0, 2).reshape(P, KC * BL)
    ).astype(dtype)


def make_in_maps(input, hx, W_ih, b_ih, W_hh, b_hh, W_c, b_c, W_hc, b_hc):
    input = np.asarray(input, np.float32)
    hx = np.asarray(hx, np.float32)
    Wq8H, Wc16H, biasH = _pack_weights(
        np.asarray(W_ih, np.float32), np.asarray(b_ih, np.float32),
        np.asarray(W_hh, np.float32), np.asarray(b_hh, np.float32),
        np.asarray(W_c, np.float32), np.asarray(b_c, np.float32),
        np.asarray(W_hc, np.float32), np.asarray(b_hc, np.float32),
    )
    in_maps = []
    for i in range(N_CORES):
        xs = input[i * BL : (i + 1) * BL]
        hs = hx[i * BL : (i + 1) * BL]
        in_maps.append(
            {
                "x8": _pack_acts(xs, E4),
                "h8": _pack_acts(hs, E4),
                "x16": _pack_acts(xs, np.float16),
                "h16": _pack_acts(hs, np.float16),
                "Wq8": Wq8H,
                "Wc16": Wc16H,
                "bias": biasH,
            }
        )
    return in_maps


def kernel(input, hx, W_ih, b_ih, W_hh, b_hh, W_c, b_c, W_hc, b_hc):
    if "nc" not in _CACHE:
        _CACHE["nc"] = _build_program()
    nc = _CACHE["nc"]

    in_maps = make_in_maps(
        input, hx, W_ih, b_ih, W_hh, b_hh, W_c, b_c, W_hc, b_hc
    )
    res = run_bass_kernel_spmd(nc, in_maps, core_ids=list(range(N_CORES)))
    out = np.empty((B, H), np.float32)
    for i, r in enumerate(res.results):
        o = (
            np.asarray(r["outT"])
            .astype(np.float32)
            .reshape(P, NJ, BL)
            .transpose(2, 1, 0)
            .reshape(BL, H)
        )
        out[i * BL : (i + 1) * BL] = o
    return out


# revision 11
# speedup vs baseline: 1.5244x; 1.0718x over previous
"""GRU-cell-variant kernel for Trainium2, data-parallel over batch on 8 cores.

Reference (per batch row b, hidden size H=1024):
    gates = sigmoid(x @ W_ih + b_ih + h @ W_hh + b_hh)   # [B, 2H]
    z, r  = gates[:, :H], gates[:, H:]
    cand  = tanh(x @ W_c + b_c + r * (h @ W_hc + b_hc))
    out   = (1 - z) * h + z * cand

Design:
  - 8-way batch shard (1024 rows/core), weights replicated. No collectives.
  - Everything on-chip is computed TRANSPOSED: out.T[o, b].
  - Warm PE streams one moving column per cycle regardless of dtype, so
    wall time ~ matmul count. fp8 DoubleRow packs K=256 per instruction
    (2 fp8 MACs/cell/cycle), halving instruction count for any matrix we
    can afford in e4m3: both gate matrices and W_hc (whose error is damped
    by sigmoid slope resp. r<1). W_c stays fp16 (tanh passes its error
    straight through; all-fp8 misses the 2e-2 budget). 448 matmuls total.
  - fp8 weights are pre-scaled by 64 on the host so the whole weight
    distribution sits in e4m3's normal range (min normal 2^-6, |w|<=2^-5);
    the activation input scale (sigmoid) resp. a folded 1/64 in the stt
    chain (candidate) undoes it for free. Measured rel err ~1.4e-2.
  - Software pipeline: iteration k runs gates(j=k) and candidate(j=k-1),
    so the cold start only needs the small fp8 operands + gate weights,
    and x16 (the big fp16 moving operand) has an extra 12us to arrive.
  - Three DMA channels: scalar HWDGE (weights), sync HWDGE (x8/x16/out),
    gpsimd SWDGE (h8/h16) — one HWDGE ring sustains only ~100GB/s, and
    iteration 0 needs ~2.6MB in ~8us.
  - ~14 dummy matmuls run during the DMA preamble so the
    PE's HAM clock gate is already at 2.4GHz when real data lands.
"""

import numpy as np
import ml_dtypes

import concourse.bass as bass
import concourse.mybir as mybir
import concourse.tile as tile
from concourse import bacc
from concourse.bass_utils import run_bass_kernel_spmd

N_CORES = 8
B = 8192
H = 1024
BL = B // N_CORES  # batch rows per core
P = 128
KC = H // P  # 8 contraction chunks of 128 per 1024-wide operand
NJ = H // P  # 8 hidden-dim tiles
WSCALE = 64.0  # fp8 weight pre-scale (undone downstream)

E4 = ml_dtypes.float8_e4m3
F8 = mybir.dt.float8e4
F16 = mybir.dt.float16
F32 = mybir.dt.float32
AF = mybir.ActivationFunctionType
ALU = mybir.AluOpType
DR = mybir.MatmulPerfMode.DoubleRow

_CACHE = {}


def _build_program():
    nc = bacc.Bacc(
        "TRN2",
        target_bir_lowering=False,
        debug=False,
        enable_asserts=False,
        num_devices=N_CORES,
    )

    # DRAM inputs, packed on the host into SBUF-friendly layouts.
    # x8/h8:   [p, kc*BL + b] = x[b, kc*128 + p]                 (e4m3)
    # x16/h16: same layout                                        (fp16)
    # Wq8: per j block of 5120 cols: [wz (2048) | wr (2048) | whc (1024)]
    #      wz/wr: [p, kc*128 + jj] = 64*Wg_full[kc*128+p, (g*8+j)*128+jj]
    #      whc:   [p, kc*128 + jj] = 64*W_hc[kc*128+p, j*128+jj]  (e4m3)
    # Wc16: [p, j*1024 + kc*128 + jj] = W_c[kc*128+p, j*128+jj]   (fp16)
    # bias: [p, 0:16] = (b_ih+b_hh)[t*128+p]; [p,16:24] = b_c;
    #       [p,24:32] = 64*b_hc
    x8 = nc.dram_tensor("x8", [P, KC * BL], F8, kind="ExternalInput").ap()
    h8 = nc.dram_tensor("h8", [P, KC * BL], F8, kind="ExternalInput").ap()
    x16 = nc.dram_tensor("x16", [P, KC * BL], F16, kind="ExternalInput").ap()
    h16 = nc.dram_tensor("h16", [P, KC * BL], F16, kind="ExternalInput").ap()
    Wq8 = nc.dram_tensor("Wq8", [P, NJ * 5632], F8, kind="ExternalInput").ap()
    Wc16 = nc.dram_tensor("Wc16", [P, NJ * 512], F16, kind="ExternalInput").ap()
    bias = nc.dram_tensor("bias", [P, 32], F32, kind="ExternalInput").ap()
    outT = nc.dram_tensor("outT", [P, NJ * BL], F16, kind="ExternalOutput").ap()

    with tile.TileContext(nc) as tc:
        with (
            tc.tile_pool(name="const", bufs=1) as cpool,
            tc.tile_pool(name="wq", bufs=3) as wqpool,
            tc.tile_pool(name="wc", bufs=2) as wcpool,
            tc.tile_pool(name="psum", bufs=8, space="PSUM") as ppool,
            tc.tile_pool(name="gates", bufs=14) as gpool,
            tc.tile_pool(name="work", bufs=14) as wpool,
        ):
            bias_sb = cpool.tile([P, 32], F32, tag="bias")
            x8_sb = cpool.tile([P, KC * BL], F8, tag="x8")
            h8_sb = cpool.tile([P, KC * BL], F8, tag="h8")
            x16_sb = cpool.tile([P, KC * BL], F16, tag="x16")
            h16_sb = cpool.tile([P, KC * BL], F16, tag="h16")
            warm = cpool.tile([P, 640], F16, tag="warm")

            # 3D views [p, kc, b] for strided chunk DMAs and DoubleRow APs
            x8s = x8_sb[:].rearrange("p (kc b) -> p kc b", kc=KC)
            h8s = h8_sb[:].rearrange("p (kc b) -> p kc b", kc=KC)
            x16s = x16_sb[:].rearrange("p (kc b) -> p kc b", kc=KC)
            h16s = h16_sb[:].rearrange("p (kc b) -> p kc b", kc=KC)
            x8d = x8.rearrange("p (kc b) -> p kc b", kc=KC)
            h8d = h8.rearrange("p (kc b) -> p kc b", kc=KC)
            x16d = x16.rearrange("p (kc b) -> p kc b", kc=KC)
            h16d = h16.rearrange("p (kc b) -> p kc b", kc=KC)

            # PE warm-up: dummy matmuls (on whatever bytes SBUF holds — the
            # psum result is never read) keep the HAM activity monitor busy
            # through the DMA preamble so real matmuls start at 2.4GHz.
            # ~8 run at the cold clock (3.4us), the rest warm; sized to end
            # right as the first real operands land (~11us).
            nc.vector.memset(warm[:], 0.0)
            pw = ppool.tile([P, 512], F32, tag="ps")
            for _ in range(16):
                nc.tensor.matmul(
                    pw[:], lhsT=warm[:, 0:128], rhs=warm[:, 128:640],
                    start=True, stop=True,
                )

            def gate_mms(pz0, pz1, w3):
                # K=2048 over [x;h] in 8 DoubleRow chunks of 256; both
                # batch halves ride each stationary chunk. x/h chunks are
                # interleaved to match the DMA arrival pattern at cold start.
                for c in (0, 4, 1, 5, 2, 6, 3, 7):
                    src3 = x8s if c < 4 else h8s
                    cc = 2 * c if c < 4 else 2 * (c - 4)
                    w = w3[:, 2 * c : 2 * c + 2, :]
                    for b, pz in ((0, pz0), (1, pz1)):
                        nc.tensor.matmul(
                            pz[:],
                            lhsT=w,
                            rhs=src3[:, cc : cc + 2, b * 512 : b * 512 + 512],
                            start=(c == 0),
                            stop=(c == 7),
                            perf_mode=DR,
                        )

            def ch_mms(ph, w3, bsl):
                # h @ W_hc for one batch half: K=1024 in 4 DoubleRow chunks
                for c in range(4):
                    nc.tensor.matmul(
                        ph[:],
                        lhsT=w3[:, 2 * c : 2 * c + 2, :],
                        rhs=h8s[:, 2 * c : 2 * c + 2, bsl],
                        start=(c == 0),
                        stop=(c == 3),
                        perf_mode=DR,
                    )

            def cx_mms(px, wc_t, jc, bsl0):
                # x @ W_c for one batch half: fp16, K=1024 in 8 chunks
                for kc in range(KC):
                    nc.tensor.matmul(
                        px[:],
                        lhsT=wc_t[:, kc * P : (kc + 1) * P],
                        rhs=x16_sb[:, kc * BL + bsl0 : kc * BL + bsl0 + 512],
                        start=(kc == 0),
                        stop=(kc == KC - 1),
                    )

            wq_tiles = {}
            wc_tiles = {}
            zs, rs, zhs = {}, {}, {}

            def eltwise(jc, b, ph, px, lo, wd):
                # candidate + blend for a [lo:lo+wd) slice of batch half b
                r_sb, z_sb, zh_sb = rs[jc][b], zs[jc][b], zhs[jc][b]
                rh = wpool.tile([P, 512], F16, tag="w")
                nc.vector.scalar_tensor_tensor(
                    rh[:, :wd], ph[:, lo : lo + wd],
                    bias_sb[:, 24 + jc : 25 + jc], r_sb[:, lo : lo + wd],
                    ALU.add, ALU.mult,
                )
                s = wpool.tile([P, 512], F16, tag="w")
                nc.vector.tensor_add(s[:, :wd], rh[:, :wd], px[:, lo : lo + wd])
                cand = wpool.tile([P, 512], F16, tag="w")
                nc.scalar.activation(
                    cand[:, :wd], s[:, :wd], AF.Tanh, scale=1.0 / WSCALE,
                    bias=bias_sb[:, 16 + jc : 17 + jc],
                )
                m = wpool.tile([P, 512], F16, tag="w")
                nc.vector.tensor_mul(
                    m[:, :wd], z_sb[:, lo : lo + wd], cand[:, :wd]
                )
                o_sb = wpool.tile([P, 512], F16, tag="w")
                nc.vector.tensor_sub(o_sb[:, :wd], m[:, :wd], zh_sb[:, lo : lo + wd])
                hoff = jc * BL + b * 512 + lo
                nc.sync.dma_start(outT[:, hoff : hoff + wd], o_sb[:, :wd])

            def load_wq(j, chunked=False):
                wq_t = wqpool.tile([P, 5632], F8, tag="wq")
                wq_tiles[j] = wq_t
                lo = j * 5632
                if chunked:
                    for c0, c1 in ((0, 1024), (1024, 2048), (2048, 3072),
                                   (3072, 4096), (4096, 5632)):
                        nc.scalar.dma_start(
                            wq_t[:, c0:c1], Wq8[:, lo + c0 : lo + c1]
                        )
                else:
                    nc.scalar.dma_start(wq_t[:], Wq8[:, lo : lo + 5632])

            def load_wc(j):
                wc_t = wcpool.tile([P, 512], F16, tag="wc")
                wc_tiles[j] = wc_t
                nc.scalar.dma_start(wc_t[:], Wc16[:, j * 512 : (j + 1) * 512])

            for k in range(NJ + 1):
                # ---- gates for j = k (+ weight/operand streaming) ----
                if k < NJ:
                    j = k
                    if k == 0:
                        # scalar ring: j0+j1 gate weights, then per-it pairs
                        load_wq(0, chunked=True)
                        nc.scalar.dma_start(bias_sb[:], bias[:])
                        load_wq(1)
                        load_wc(0)
                        # sync ring: x8 chunks, then x16 batch-half-major
                        nc.sync.dma_start(x8s[:, 0:2, :], x8d[:, 0:2, :])
                        nc.sync.dma_start(x8s[:, 2:4, :], x8d[:, 2:4, :])
                        nc.sync.dma_start(x8s[:, 4:6, :], x8d[:, 4:6, :])
                        nc.sync.dma_start(x8s[:, 6:8, :], x8d[:, 6:8, :])
                        nc.sync.dma_start(x16s[:, 4:8, 0:512], x16d[:, 4:8, 0:512])
                        nc.sync.dma_start(x16s[:, 4:8, 512:1024], x16d[:, 4:8, 512:1024])
                        # gpsimd SWDGE ring: h8 chunks, then h16 per-j stream
                        nc.gpsimd.dma_start(h8s[:, 0:2, :], h8d[:, 0:2, :])
                        nc.gpsimd.dma_start(h8s[:, 2:4, :], h8d[:, 2:4, :])
                        nc.gpsimd.dma_start(h8s[:, 4:6, :], h8d[:, 4:6, :])
                        nc.gpsimd.dma_start(h8s[:, 6:8, :], h8d[:, 6:8, :])
                        nc.gpsimd.dma_start(h16s[:, 0, :], h16d[:, 0, :])
                    else:
                        if k + 1 < NJ:
                            load_wq(k + 1)
                        load_wc(k)
                        nc.gpsimd.dma_start(h16s[:, j, :], h16d[:, j, :])
                    wq_t = wq_tiles[j]

                    wz3 = wq_t[:, 0:2048].rearrange("p (kc o) -> p kc o", kc=16)
                    wr3 = wq_t[:, 2048:4096].rearrange("p (kc o) -> p kc o", kc=16)

                    pz0 = ppool.tile([P, 512], F32, tag="ps")
                    pz1 = ppool.tile([P, 512], F32, tag="ps")
                    gate_mms(pz0, pz1, wz3)
                    zs[j], zhs[j] = [], []
                    for b, pz in ((0, pz0), (1, pz1)):
                        z_sb = gpool.tile([P, 512], F16, tag="g")
                        nc.scalar.activation(
                            z_sb[:], pz[:], AF.Sigmoid,
                            bias=bias_sb[:, j : j + 1], scale=1.0 / WSCALE,
                        )
                        hsl = h16_sb[:, j * BL + b * 512 : j * BL + b * 512 + 512]
                        zh_sb = gpool.tile([P, 512], F16, tag="g")
                        nc.vector.scalar_tensor_tensor(
                            zh_sb[:], z_sb[:], 1.0, hsl, ALU.subtract, ALU.mult
                        )
                        zs[j].append(z_sb)
                        zhs[j].append(zh_sb)

                    pr0 = ppool.tile([P, 512], F32, tag="ps")
                    pr1 = ppool.tile([P, 512], F32, tag="ps")
                    gate_mms(pr0, pr1, wr3)
                    rs[j] = []
                    for b, pr in ((0, pr0), (1, pr1)):
                        r_sb = gpool.tile([P, 512], F16, tag="g")
                        nc.scalar.activation(
                            r_sb[:], pr[:], AF.Sigmoid,
                            bias=bias_sb[:, 8 + j : 9 + j], scale=1.0 / WSCALE,
                        )
                        rs[j].append(r_sb)

                # ---- candidate + blend for jc = k-1 ----
                if k >= 1:
                    jc = k - 1
                    whc3 = wq_tiles[jc][:, 4096:5120].rearrange(
                        "p (kc o) -> p kc o", kc=KC
                    )
                    wc_t = wc_tiles[jc]
                    if k < NJ:
                        # interleave both halves (shared stationary chunks)
                        ph0 = ppool.tile([P, 512], F32, tag="ps")
                        ph1 = ppool.tile([P, 512], F32, tag="ps")
                        for c in range(4):
                            w = whc3[:, 2 * c : 2 * c + 2, :]
                            for b, ph in ((0, ph0), (1, ph1)):
                                nc.tensor.matmul(
                                    ph[:], lhsT=w,
                                    rhs=h8s[:, 2 * c : 2 * c + 2,
                                            b * 512 : b * 512 + 512],
                                    start=(c == 0), stop=(c == 3),
                                    perf_mode=DR,
                                )
                        px0 = ppool.tile([P, 512], F32, tag="ps")
                        px1 = ppool.tile([P, 512], F32, tag="ps")
                        wc8_3 = wq_tiles[jc][:, 5120:5632].rearrange(
                            "p (kc o) -> p kc o", kc=4
                        )
                        for c in range(2):
                            w = wc8_3[:, 2 * c : 2 * c + 2, :]
                            for b, px in ((0, px0), (1, px1)):
                                nc.tensor.matmul(
                                    px[:], lhsT=w,
                                    rhs=x8s[:, 2 * c : 2 * c + 2,
                                            b * 512 : b * 512 + 512],
                                    start=(c == 0), stop=False, perf_mode=DR,
                                )
                        for kc in range(4, KC):
                            w = wc_t[:, (kc - 4) * P : (kc - 3) * P]
                            for b, px in ((0, px0), (1, px1)):
                                nc.tensor.matmul(
                                    px[:], lhsT=w,
                                    rhs=x16_sb[:, kc * BL + b * 512 :
                                               kc * BL + b * 512 + 512],
                                    start=False, stop=(kc == KC - 1),
                                )
                        eltwise(jc, 0, ph0, px0, 0, 512)
                        eltwise(jc, 1, ph1, px1, 0, 512)
                    else:
                        # final iteration: 256-wide batch quarters so each
                        # quarter's eltwise chain hides under the next
                        # quarter's matmuls, and the last out-DMA (whose HBM
                        # write-completion gates the end barrier) issues as
                        # soon after the last matmul as possible. m/o run on
                        # the otherwise-idle GpSimd to unclog the DVE queue.
                        for q in range(4):
                            b, lo = q // 2, (q % 2) * 256
                            qs = b * 512 + lo
                            ph_t = ppool.tile([P, 512], F32, tag="ps")
                            ph = ph_t[:, 0:256]
                            for c in range(4):
                                nc.tensor.matmul(
                                    ph[:], lhsT=whc3[:, 2 * c : 2 * c + 2, :],
                                    rhs=h8s[:, 2 * c : 2 * c + 2, qs : qs + 256],
                                    start=(c == 0), stop=(c == 3), perf_mode=DR,
                                )
                            px_t = ppool.tile([P, 512], F32, tag="ps")
                            px = px_t[:, 0:256]
                            wc8_3 = wq_tiles[jc][:, 5120:5632].rearrange(
                                "p (kc o) -> p kc o", kc=4
                            )
                            for c in range(2):
                                nc.tensor.matmul(
                                    px[:], lhsT=wc8_3[:, 2 * c : 2 * c + 2, :],
                                    rhs=x8s[:, 2 * c : 2 * c + 2, qs : qs + 256],
                                    start=(c == 0), stop=False, perf_mode=DR,
                                )
                            for kc in range(4, KC):
                                nc.tensor.matmul(
                                    px[:], lhsT=wc_t[:, (kc - 4) * P : (kc - 3) * P],
                                    rhs=x16_sb[:, kc * BL + qs : kc * BL + qs + 256],
                                    start=False, stop=(kc == KC - 1),
                                )
                            r_sb, z_sb, zh_sb = rs[jc][b], zs[jc][b], zhs[jc][b]
                            rh = wpool.tile([P, 512], F16, tag="w")
                            nc.vector.scalar_tensor_tensor(
                                rh[:, :256], ph[:],
                                bias_sb[:, 24 + jc : 25 + jc],
                                r_sb[:, lo : lo + 256], ALU.add, ALU.mult,
                            )
                            s = wpool.tile([P, 512], F16, tag="w")
                            nc.vector.tensor_add(s[:, :256], rh[:, :256], px[:])
                            cand = wpool.tile([P, 512], F16, tag="w")
                            nc.scalar.activation(
                                cand[:, :256], s[:, :256], AF.Tanh,
                                scale=1.0 / WSCALE,
                                bias=bias_sb[:, 16 + jc : 17 + jc],
                            )
                            m = wpool.tile([P, 512], F16, tag="w")
                            nc.gpsimd.tensor_mul(
                                m[:, :256], z_sb[:, lo : lo + 256], cand[:, :256]
                            )
                            o_sb = wpool.tile([P, 512], F16, tag="w")
                            nc.gpsimd.tensor_sub(
                                o_sb[:, :256], m[:, :256], zh_sb[:, lo : lo + 256]
                            )
                            hoff = jc * BL + qs
                            nc.sync.dma_start(
                                outT[:, hoff : hoff + 256], o_sb[:, :256]
                            )

    nc.compile()
    return nc


def _pack_weights(W_ih, b_ih, W_hh, b_hh, W_c, b_c, W_hc, b_hc):
    Wg_full = np.concatenate([W_ih, W_hh], axis=0)  # [2H, 2H] = [k, o]
    # [kc, p, t, jj] -> [p, t, kc, jj]
    wg = Wg_full.reshape(16, P, 16, P).transpose(1, 2, 0, 3)
    whc = W_hc.reshape(KC, P, NJ, P).transpose(1, 2, 0, 3)
    wc = W_c.reshape(KC, P, NJ, P).transpose(1, 2, 0, 3)  # [p, j, kc, jj]
    Wq8H = np.concatenate(
        [
            np.concatenate(
                [
                    wg[:, j].reshape(P, 2048),
                    wg[:, 8 + j].reshape(P, 2048),
                    whc[:, j].reshape(P, 1024),
                    wc[:, j, 0:4].reshape(P, 512),
                ],
                axis=1,
            )
            for j in range(NJ)
        ],
        axis=1,
    )
    Wq8H = np.ascontiguousarray(Wq8H * WSCALE).astype(E4)
    Wc16H = np.ascontiguousarray(
        WSCALE
        * np.concatenate([wc[:, j, 4:8].reshape(P, 512) for j in range(NJ)], axis=1)
    ).astype(np.float16)
    biasH = np.empty((P, 32), np.float32)
    biasH[:, 0:16] = (b_ih + b_hh).reshape(16, P).T
    biasH[:, 16:24] = b_c.reshape(NJ, P).T
    biasH[:, 24:32] = WSCALE * b_hc.reshape(NJ, P).T
    return Wq8H, Wc16H, np.ascontiguousarray(biasH)


def _pack_acts(a, dtype):
    # [BL, H] -> [p, kc*BL + b] with a[b, kc*128+p]
    return np.ascontiguousarray(
        a.T.reshape(KC, P, BL).transpose(1, 0, 2).reshape(P, KC * BL)
    ).astype(dtype)


def make_in_maps(input, hx, W_ih, b_ih, W_hh, b_hh, W_c, b_c, W_hc, b_hc):
    input = np.asarray(input, np.float32)
    hx = np.asarray(hx, np.float32)
    Wq8H, Wc16H, biasH = _pack_weights(
        np.asarray(W_ih, np.float32), np.asarray(b_ih, np.float32),
        np.asarray(W_hh, np.float32), np.asarray(b_hh, np.float32),
        np.asarray(W_c, np.float32), np.asarray(b_c, np.float32),
        np.asarray(W_hc, np.float32), np.asarray(b_hc, np.float32),
    )
    in_maps = []
    for i in range(N_CORES):
        xs = input[i * BL : (i + 1) * BL]
        hs = hx[i * BL : (i + 1) * BL]
        in_maps.append(
            {
                "x8": _pack_acts(xs, E4),
                "h8": _pack_acts(hs, E4),
                "x16": _pack_acts(xs, np.float16),
                "h16": _pack_acts(hs, np.float16),
                "Wq8": Wq8H,
                "Wc16": Wc16H,
                "bias": biasH,
            }
        )
    return in_maps


def kernel(input, hx, W_ih, b_ih, W_hh, b_hh, W_c, b_c, W_hc, b_hc):
    if "nc" not in _CACHE:
        _CACHE["nc"] = _build_program()
    nc = _CACHE["nc"]

    in_maps = make_in_maps(
        input, hx, W_ih, b_ih, W_hh, b_hh, W_c, b_c, W_hc, b_hc
    )
    res = run_bass_kernel_spmd(nc, in_maps, core_ids=list(range(N_CORES)))
    out = np.empty((B, H), np.float32)
    for i, r in enumerate(res.results):
        o = (
            np.asarray(r["outT"])
            .astype(np.float32)
            .reshape(P, NJ, BL)
            .transpose(2, 1, 0)
            .reshape(BL, H)
        )
        out[i * BL : (i + 1) * BL] = o
    return out
